# revision 1
# baseline (speedup 1.0000x reference)
"""3-layer GCN (GCNConv x3 + leaky_relu + first-node-per-graph readout) on
8 Trainium2 NeuronCores via Bass/Tile.

Strategy (graph-partitioned, aggregate-then-transform):
  - Nodes are partitioned contiguously across 8 cores (2500 each); edges are
    owned by their destination core. Weights are replicated.
  - GCN normalization is factored: norm[e] = dis[src]*dis[dst] with
    dis = deg^-1/2, so each layer becomes
        out = dis * segsum_dst( (dis*h)[src] ) @ W + b
    The "message table" (dis*h) is stored node-major in DRAM as bf16 and
    gathered per-edge with dma_gather; tables are exchanged between layers
    with an AllGather collective.
  - Per destination window of 128 nodes, edges are processed in chunks of
    128: a one-hot selection matrix S[e, slot(dst_e)] = 1 is built on the
    vector engine (iota + is_equal) and the segment-sum becomes a PE matmul
    accumulating into PSUM:  aggF[c, d] += gathered[e, c]^T @ S[e, d].
  - Layer 3 only needs the 100 first-nodes of each graph, so only edges
    whose dst is a graph's first node are processed (~1.6k edges total).

kernel(**inputs) takes the full unsharded inputs and returns the full
[n_graphs, 32] float32 output.
"""

import sys

sys.path.insert(0, "/opt/trn_rl_repo")

import numpy as np

import concourse.bacc as bacc
import concourse.mybir as mybir
import concourse.tile as tile
from concourse.bass_utils import run_bass_kernel_spmd

F32 = mybir.dt.float32
BF16 = mybir.dt.bfloat16
I16 = mybir.dt.int16

N_CORES = 8
C0, C1, C2, C3 = 128, 256, 256, 32
ZPAD = 64  # z-table row padded to 64 f32 (256B, dma_gather elem granularity)

# ---------------------------------------------------------------------------
# Host-side prep: degrees/normalization, edge partitioning, index layouts
# ---------------------------------------------------------------------------


def _pack_gather_idx(idx, n_slots):
    """int32 row indices -> dma_gather int16 layout [128, n_slots//16].

    dma_gather reads index j from partition j%16, column j//16 (partitions
    16..127 are replicas for the 8 Q7 cores)."""
    assert n_slots % 16 == 0
    a = np.zeros(n_slots, np.int16)
    a[: len(idx)] = idx.astype(np.int16)
    a = a.reshape(n_slots // 16, 16).T  # [16, cols]
    return np.tile(a, (8, 1))  # [128, cols]


def _pack_chunked(vals, n_slots, fill):
    """values per edge -> [128, n_slots//128] layout (edge j at [j%128, j//128])."""
    a = np.full(n_slots, fill, np.float32)
    a[: len(vals)] = vals
    return a.reshape(n_slots // 128, 128).T.copy()  # [128, chunks]


def host_prep(x, src, dst, batch, W1, b1, W2, b2, W3, b3, n_graphs):
    N = x.shape[0]
    G = int(n_graphs)
    E = len(src)
    NPC = N // N_CORES
    W = (NPC + 127) // 128
    NPAD = W * 128

    deg = np.bincount(dst, minlength=N).astype(np.float32)
    dis = np.where(deg > 0, 1.0 / np.sqrt(deg), 0.0).astype(np.float32)

    first = np.full(G, N, np.int64)
    np.minimum.at(first, batch.astype(np.int64), np.arange(N))

    owner = dst // NPC
    local = dst - owner * NPC
    win = local // 128
    slot = local % 128
    remap_src = (src // NPC) * NPAD + (src % NPC)

    # group edges by (core, window)
    order = np.argsort(owner * W + win, kind="stable")
    counts = np.bincount(owner * W + win, minlength=N_CORES * W).reshape(
        N_CORES, W
    )
    P = max(1, int(np.ceil(counts.max() / 128)))
    NS = P * 128

    # layer-3 edges: dst is a first node
    is_first = np.zeros(N, bool)
    is_first[first] = True
    gid_of_first = np.full(N, -1, np.int64)
    gid_of_first[first] = np.arange(G)
    graphs_per_core = [
        np.nonzero((first >= i * NPC) & (first < (i + 1) * NPC))[0]
        for i in range(N_CORES)
    ]
    gslot = np.full(G, -1, np.int64)  # slot of graph within its core
    for i in range(N_CORES):
        gslot[graphs_per_core[i]] = np.arange(len(graphs_per_core[i]))
    e3 = np.nonzero(is_first[dst])[0]
    e3_owner = dst[e3] // NPC
    cnt3 = np.bincount(e3_owner, minlength=N_CORES)
    P3 = max(1, int(np.ceil(cnt3.max() / 128)))
    NS3 = P3 * 128

    W2r = np.ascontiguousarray(
        np.concatenate([W2[0:128, :], W2[128:256, :]], axis=1)
    )  # [128, 512]
    W3r = np.ascontiguousarray(
        np.concatenate([W3[0:128, :], W3[128:256, :]], axis=1)
    )  # [128, 64]
    b3p = np.zeros(ZPAD, np.float32)
    b3p[:C3] = b3

    in_maps = []
    ptr = np.concatenate([[0], np.cumsum(counts.ravel())])
    for i in range(N_CORES):
        xs = np.zeros((NPAD, C0), np.float32)
        xs[:NPC] = x[i * NPC : (i + 1) * NPC]
        diso = np.zeros(NPAD, np.float32)
        diso[:NPC] = dis[i * NPC : (i + 1) * NPC]
        disw = diso.reshape(W, 128).T.copy()  # [128, W]
        disbc = np.tile(diso[None, :], (128, 1))  # [128, NPAD]

        idx_l = np.zeros((W, 128, NS // 16), np.int16)
        slot_l = np.zeros((W, 128, P), np.float32)
        for w in range(W):
            k = i * W + w
            ee = order[ptr[k] : ptr[k + 1]]
            idx_l[w] = _pack_gather_idx(remap_src[ee], NS)
            slot_l[w] = _pack_chunked(slot[ee].astype(np.float32), NS, -1.0)

        ee3 = e3[e3_owner == i]
        idx3 = _pack_gather_idx(remap_src[ee3], NS3)
        slot3 = _pack_chunked(gslot[gid_of_first[dst[ee3]]], NS3, -1.0)
        disf = np.zeros((128, 1), np.float32)
        gl = graphs_per_core[i]
        disf[: len(gl), 0] = dis[first[gl]]

        in_maps.append(
            {
                "x": xs,
                "idx": idx_l,
                "slot": slot_l,
                "idx3": idx3,
                "slot3": slot3,
                "disw": disw,
                "disbc": disbc,
                "disf": disf,
                "w1": np.ascontiguousarray(W1),
                "w2r": W2r,
                "w3r": W3r,
                "iotaf": np.tile(np.arange(128, dtype=np.float32)[None, :], (128, 1)),
                "b1bc": np.tile(b1[None, :], (128, 1)),
                "b2c": b2.reshape(2, 128).T.copy(),
                "b3bc": np.tile(b3p[None, :], (128, 1)),
            }
        )

    meta = dict(
        N=N, G=G, NPC=NPC, W=W, NPAD=NPAD, P=P, P3=P3,
        graphs_per_core=graphs_per_core,
    )
    return in_maps, meta


# ---------------------------------------------------------------------------
# Device program
# ---------------------------------------------------------------------------


def build_program(meta, compile_=True, repeat=1):
    W, NPAD, P, P3 = meta["W"], meta["NPAD"], meta["P"], meta["P3"]
    NS, NS3 = P * 128, P3 * 128

    nc = bacc.Bacc(
        "TRN2", target_bir_lowering=False, debug=False, num_devices=N_CORES
    )
    dp = nc.declare_dram_parameter
    x_d = dp("x", [NPAD, C0], F32, isOutput=False)
    idx_d = dp("idx", [W, 128, NS // 16], I16, isOutput=False)
    slot_d = dp("slot", [W, 128, P], F32, isOutput=False)
    idx3_d = dp("idx3", [128, NS3 // 16], I16, isOutput=False)
    slot3_d = dp("slot3", [128, P3], F32, isOutput=False)
    disw_d = dp("disw", [128, W], F32, isOutput=False)
    disbc_d = dp("disbc", [128, NPAD], F32, isOutput=False)
    disf_d = dp("disf", [128, 1], F32, isOutput=False)
    w1_d = dp("w1", [128, C1], F32, isOutput=False)
    w2r_d = dp("w2r", [128, 2 * C2], F32, isOutput=False)
    w3r_d = dp("w3r", [128, 2 * C3], F32, isOutput=False)
    iotaf_d = dp("iotaf", [128, 128], F32, isOutput=False)
    b1bc_d = dp("b1bc", [128, C1], F32, isOutput=False)
    b2c_d = dp("b2c", [128, 2], F32, isOutput=False)
    b3bc_d = dp("b3bc", [128, ZPAD], F32, isOutput=False)
    out_d = dp("out", [128, ZPAD], F32, isOutput=True)

    rg = [list(range(N_CORES))]
    AL = mybir.AluOpType

    with tile.TileContext(nc) as tc:
        with (
            tc.tile_pool(name="const", bufs=1) as cpool,
            tc.tile_pool(name="work", bufs=4) as pool,
            tc.tile_pool(name="gath", bufs=4) as gpool,
            tc.tile_pool(name="psum", bufs=2, space="PSUM") as psum,
            tc.tile_pool(name="dram", bufs=1, space="DRAM") as dram,
        ):
            # ---- constants ----
            disw = cpool.tile([128, W], F32)
            nc.sync.dma_start(out=disw[:], in_=disw_d[:, :])
            disbc = cpool.tile([128, NPAD], F32)
            nc.sync.dma_start(out=disbc[:], in_=disbc_d[:, :])
            disf = cpool.tile([128, 1], F32)
            nc.sync.dma_start(out=disf[:], in_=disf_d[:, :])
            w1 = cpool.tile([128, C1], F32)
            nc.sync.dma_start(out=w1[:], in_=w1_d[:, :])
            w2r = cpool.tile([128, 2 * C2], F32)
            nc.sync.dma_start(out=w2r[:], in_=w2r_d[:, :])
            w3r = cpool.tile([128, 2 * C3], F32)
            nc.sync.dma_start(out=w3r[:], in_=w3r_d[:, :])
            b1bc = cpool.tile([128, C1], F32)
            nc.sync.dma_start(out=b1bc[:], in_=b1bc_d[:, :])
            b2c = cpool.tile([128, 2], F32)
            nc.sync.dma_start(out=b2c[:], in_=b2c_d[:, :])
            b3bc = cpool.tile([128, ZPAD], F32)
            nc.sync.dma_start(out=b3bc[:], in_=b3bc_d[:, :])

            iota_f = cpool.tile([128, 128], F32)
            nc.sync.dma_start(out=iota_f[:], in_=iotaf_d[:, :])

            # ---- stage A: x-table = bf16(dis * x), AllGather ----
            for _rep in range(repeat):
              # DRAM tables (per repetition: Shared outputs need 1 writer)
              xt_in = dram.tile([NPAD, C0], BF16)
              xt_full = dram.tile([N_CORES * NPAD, C0], BF16, addr_space="Shared")
              h1_in = dram.tile([NPAD, C1], BF16)
              h1_full = dram.tile([N_CORES * NPAD, C1], BF16, addr_space="Shared")
              z_in = dram.tile([NPAD, ZPAD], F32)
              z_full = dram.tile([N_CORES * NPAD, ZPAD], F32, addr_space="Shared")
              for w in range(W):
                  xs = pool.tile([128, C0], F32, tag="xs")
                  nc.sync.dma_start(out=xs[:], in_=x_d[w * 128 : (w + 1) * 128, :])
                  xt = pool.tile([128, C0], BF16, tag="xt")
                  nc.vector.tensor_scalar(
                      xt[:], xs[:], disw[:, w : w + 1], None, AL.mult
                  )
                  nc.sync.dma_start(out=xt_in[w * 128 : (w + 1) * 128, :], in_=xt[:])
              nc.gpsimd.collective_compute(
                  "AllGather", AL.bypass, replica_groups=rg,
                  ins=[xt_in.opt()], outs=[xt_full.opt()],
              )

              def build_S(slot_sb, n_chunks, dt, tag):
                  S = pool.tile([128, n_chunks * 128], dt, tag=tag)
                  for c in range(n_chunks):
                      nc.vector.tensor_scalar(
                          S[:, c * 128 : (c + 1) * 128],
                          iota_f[:],
                          slot_sb[:, c : c + 1],
                          None,
                          AL.is_equal,
                      )
                  return S

              def aggregate(w, table, Cin):
                  """gather + one-hot scatter matmul -> SBUF aggF [Cin, 128dsts]"""
                  idx_sb = pool.tile([128, NS // 16], I16, tag="idx")
                  nc.sync.dma_start(out=idx_sb[:], in_=idx_d[w, :, :])
                  slot_sb = pool.tile([128, P], F32, tag="slot")
                  nc.sync.dma_start(out=slot_sb[:], in_=slot_d[w, :, :])
                  g = gpool.tile([128, P, Cin], BF16, tag="gath")
                  # one dma_gather handles at most 1024 indices (8 chunks)
                  for s0 in range(0, P, 8):
                      cs = min(8, P - s0)
                      nc.gpsimd.dma_gather(
                          g[:, s0 : s0 + cs, :],
                          table[:, :],
                          idx_sb[:, s0 * 8 : (s0 + cs) * 8],
                          num_idxs=cs * 128,
                          num_idxs_reg=cs * 128,
                          elem_size=Cin,
                      )
                  S = build_S(slot_sb, P, BF16, "S")
                  nh = Cin // 128
                  aggp = psum.tile([128, Cin], F32, tag="agg")
                  for h in range(nh):
                      for c in range(P):
                          nc.tensor.matmul(
                              aggp[:, h * 128 : (h + 1) * 128],
                              lhsT=g[:, c, h * 128 : (h + 1) * 128],
                              rhs=S[:, c * 128 : (c + 1) * 128],
                              start=(c == 0),
                              stop=(c == P - 1),
                          )
                  agg = pool.tile([128, Cin], F32, tag="aggsb")
                  nc.vector.tensor_copy(agg[:], aggp[:])
                  return agg

              # ---- layer 1 ----
              for w in range(W):
                  agg = aggregate(w, xt_full, C0)
                  h1p = psum.tile([128, C1], F32, tag="dense")
                  nc.tensor.matmul(
                      h1p[:], lhsT=agg[:, 0:128], rhs=w1[:], start=True, stop=True
                  )
                  u = pool.tile([128, C1], F32, tag="u")
                  nc.vector.tensor_scalar(
                      u[:], h1p[:], disw[:, w : w + 1], None, AL.mult
                  )
                  nc.vector.tensor_tensor(u[:], u[:], b1bc[:], op=AL.add)
                  v = pool.tile([128, C1], F32, tag="v")
                  nc.scalar.activation(
                      v[:], u[:], mybir.ActivationFunctionType.Copy, scale=0.01
                  )
                  nc.vector.tensor_tensor(u[:], u[:], v[:], op=AL.max)
                  t1 = pool.tile([128, C1], BF16, tag="t1")
                  nc.vector.tensor_scalar(
                      t1[:], u[:], disw[:, w : w + 1], None, AL.mult
                  )
                  nc.sync.dma_start(
                      out=h1_in[w * 128 : (w + 1) * 128, :], in_=t1[:]
                  )
              nc.gpsimd.collective_compute(
                  "AllGather", AL.bypass, replica_groups=rg,
                  ins=[h1_in.opt()], outs=[h1_full.opt()],
              )

              # ---- layer 2 (+ z = dis * (h2 @ W3)) ----
              for w in range(W):
                  agg = aggregate(w, h1_full, C1)
                  h2p = psum.tile([128, C2], F32, tag="dense")
                  for m in range(2):
                      for k in range(2):
                          nc.tensor.matmul(
                              h2p[:, m * 128 : (m + 1) * 128],
                              lhsT=w2r[:, k * 256 + m * 128 : k * 256 + (m + 1) * 128],
                              rhs=agg[:, k * 128 : (k + 1) * 128],
                              start=(k == 0),
                              stop=(k == 1),
                          )
                  h2 = pool.tile([128, C2], F32, tag="u")
                  for m in range(2):
                      sl = slice(m * 128, (m + 1) * 128)
                      nc.vector.tensor_tensor(
                          h2[:, sl], h2p[:, sl],
                          disbc[:, w * 128 : (w + 1) * 128], op=AL.mult,
                      )
                      nc.vector.tensor_scalar(
                          h2[:, sl], h2[:, sl], b2c[:, m : m + 1], None, AL.add
                      )
                  v = pool.tile([128, C2], F32, tag="v")
                  nc.scalar.activation(
                      v[:], h2[:], mybir.ActivationFunctionType.Copy, scale=0.01
                  )
                  nc.vector.tensor_tensor(h2[:], h2[:], v[:], op=AL.max)
                  zp = psum.tile([128, ZPAD], F32, tag="z")
                  for k in range(2):
                      nc.tensor.matmul(
                          zp[:, 0:C3],
                          lhsT=h2[:, k * 128 : (k + 1) * 128],
                          rhs=w3r[:, k * C3 : (k + 1) * C3],
                          start=(k == 0),
                          stop=(k == 1),
                      )
                  zt = pool.tile([128, ZPAD], F32, tag="zt")
                  nc.vector.memset(zt[:, C3:ZPAD], 0.0)
                  nc.vector.tensor_scalar(
                      zt[:, 0:C3], zp[:, 0:C3], disw[:, w : w + 1], None, AL.mult
                  )
                  nc.sync.dma_start(
                      out=z_in[w * 128 : (w + 1) * 128, :], in_=zt[:]
                  )
              nc.gpsimd.collective_compute(
                  "AllGather", AL.bypass, replica_groups=rg,
                  ins=[z_in.opt()], outs=[z_full.opt()],
              )

              # ---- layer 3: only first-node dsts ----
              idx3_sb = pool.tile([128, NS3 // 16], I16, tag="idx")
              nc.sync.dma_start(out=idx3_sb[:], in_=idx3_d[:, :])
              slot3_sb = pool.tile([128, P3], F32, tag="slot3")
              nc.sync.dma_start(out=slot3_sb[:], in_=slot3_d[:, :])
              g3 = gpool.tile([128, P3, ZPAD], F32, tag="g3")
              for s0 in range(0, P3, 8):
                  cs = min(8, P3 - s0)
                  nc.gpsimd.dma_gather(
                      g3[:, s0 : s0 + cs, :],
                      z_full[:, :],
                      idx3_sb[:, s0 * 8 : (s0 + cs) * 8],
                      num_idxs=cs * 128,
                      num_idxs_reg=cs * 128,
                      elem_size=ZPAD,
                  )
              S3 = build_S(slot3_sb, P3, F32, "S3")
              op = psum.tile([128, ZPAD], F32, tag="z")
              for c in range(P3):
                  nc.tensor.matmul(
                      op[:],
                      lhsT=S3[:, c * 128 : (c + 1) * 128],
                      rhs=g3[:, c, :],
                      start=(c == 0),
                      stop=(c == P3 - 1),
                  )
              outt = pool.tile([128, ZPAD], F32, tag="outt")
              nc.vector.tensor_scalar(outt[:], op[:], disf[:, 0:1], None, AL.mult)
              nc.vector.tensor_tensor(outt[:], outt[:], b3bc[:], op=AL.add)
              nc.sync.dma_start(out=out_d[:, :], in_=outt[:])

    if compile_:
        nc.compile()
    return nc


# ---------------------------------------------------------------------------
# Entry point
# ---------------------------------------------------------------------------

_cache = {}


def _prepare(inputs):
    in_maps, meta = host_prep(**inputs)
    key = (meta["W"], meta["NPAD"], meta["P"], meta["P3"])
    if key not in _cache:
        _cache[key] = build_program(meta)
    return _cache[key], in_maps, meta


def assemble_output(results, meta):
    G = meta["G"]
    out = np.zeros((G, C3), np.float32)
    for i in range(N_CORES):
        gl = meta["graphs_per_core"][i]
        if len(gl):
            out[gl] = results[i]["out"][: len(gl), :C3]
    return out


def kernel(**inputs):
    nc, in_maps, meta = _prepare(inputs)
    res = run_bass_kernel_spmd(
        nc, in_maps, core_ids=list(range(N_CORES))
    )
    return assemble_output(res.results, meta)


if __name__ == "__main__":
    # smoke test with random data shaped like the real problem
    rng = np.random.default_rng(0)
    N, E, G = 20000, 320000, 100
    inputs = dict(
        x=rng.standard_normal((N, 128), dtype=np.float32),
        src=rng.integers(0, N, E).astype(np.int32),
        dst=rng.integers(0, N, E).astype(np.int32),
        batch=(np.arange(N) // (N // G)).astype(np.int32),
        W1=rng.standard_normal((128, 256), dtype=np.float32),
        b1=rng.standard_normal(256).astype(np.float32),
        W2=rng.standard_normal((256, 256), dtype=np.float32),
        b2=rng.standard_normal(256).astype(np.float32),
        W3=rng.standard_normal((256, 32), dtype=np.float32),
        b3=rng.standard_normal(32).astype(np.float32),
        n_graphs=G,
    )
    out = kernel(**inputs)
    print("out", out.shape, out.dtype, float(np.abs(out).max()))

